# revision 43
# baseline (speedup 1.0000x reference)
"""Self-contained Trainium2 Bass kernel for causal GQA self-attention.

Problem (hardcoded): B=2, T=2048, D=2048, H=16 Q-heads, KV=4 kv-heads,
hd=128, rotate-half RoPE (theta=10000), causal softmax, out-projection.

Distribution over 8 NeuronCores (tensor parallel over heads):
 - core c owns Q heads {2c, 2c+1} and KV head c//2 (each KV head is shared
   by two cores, so the K/V projection is computed twice - cheaper than a
   collective at the start).
 - each core computes q/k/v projections + RoPE + causal attention for its
   heads over BOTH batch rows, entirely locally (bf16 matmuls, f32 softmax).
 - an 8-way AllToAll (one per local head) reshards the attention outputs:
   afterwards core c holds all 16 heads' outputs for its slice of the
   flattened [B*T] row space (rows [512c, 512c+512)).
 - each core computes the final out-projection for its rows with the full
   Wo and returns its [512, 2048] slice; the host concatenates the slices.

Schedule notes (v2):
 - proj0 runs kd-outer so matmuls start as soon as the first x/w k-slices
   arrive from HBM instead of waiting for the full 10MB load.
 - S-score blocks are computed in pairs into [128,1024] PSUM tiles and
   exp'd with one activation instruction per pair (halves ACT overhead).
   Diagonal blocks are computed full-width; the causal mask multiply
   zeroes the upper triangle.
 - out-projection accumulates all 16 head-blocks in PSUM in a single
   pass (no DRAM read-accumulate); the first 8 output tiles accumulate
   their h=0 half while the second AllToAll is still in flight.
"""

import numpy as np
import ml_dtypes

import concourse.bass as bass
import concourse.tile as tile
from concourse import bacc, mybir
from concourse.bass_utils import run_bass_kernel_spmd

BF = mybir.dt.bfloat16
F32 = mybir.dt.float32

B, T, D = 2, 2048, 2048
H, KVH, HD = 16, 4, 128
THETA = 10000.0
NCORES = 8
TT = T // 128           # 16 t-tiles
KD = D // 128           # 16 contraction tiles

_compiled = None
PHASE_MARKS = []
DEBUG_KV = False

# test-harness knobs (not used by the grading path)
TRACE = False
TRACE_DIR = None
LAST_RESULT = None


def _build():
    nc = bacc.Bacc(
        "TRN2", target_bir_lowering=False, debug=False, num_devices=NCORES
    )

    # ---- I/O ----
    xt0 = nc.dram_tensor("xt0", [D, T], BF, kind="ExternalInput").ap()
    xt1 = nc.dram_tensor("xt1", [D, T], BF, kind="ExternalInput").ap()
    wqkv = nc.dram_tensor("wqkv", [D, 512], BF, kind="ExternalInput").ap()
    wo = nc.dram_tensor("wo", [D, D], BF, kind="ExternalInput").ap()
    cosq = nc.dram_tensor("cosq", [128, TT * 64], BF, kind="ExternalInput").ap()
    sinq = nc.dram_tensor("sinq", [128, TT * 64], BF, kind="ExternalInput").ap()
    cosk = nc.dram_tensor("cosk", [128, TT * 64], BF, kind="ExternalInput").ap()
    sink = nc.dram_tensor("sink", [128, TT * 64], BF, kind="ExternalInput").ap()
    maskq = nc.dram_tensor("maskq", [128, 2048], BF, kind="ExternalInput").ap()
    identin = nc.dram_tensor("identin", [128, 128], BF, kind="ExternalInput").ap()
    out_ext = nc.dram_tensor("out", [512, D], F32, kind="ExternalOutput").ap()
    dbg_ext = (
        nc.dram_tensor("dbg", [128, TT * 256], BF, kind="ExternalOutput").ap()
        if DEBUG_KV
        else None
    )

    xts = [xt0, xt1]

    with tile.TileContext(nc) as tc:
        with (
            tc.tile_pool(name="const", bufs=1) as constp,
            tc.tile_pool(name="big", bufs=1) as bigp,
            tc.tile_pool(name="persist", bufs=1) as persist,
            tc.tile_pool(name="work", bufs=2) as work,
            tc.tile_pool(name="combp", bufs=1) as combp,
            tc.tile_pool(name="pbp", bufs=10) as pbp,
            tc.tile_pool(name="aop", bufs=16) as aop,
            tc.tile_pool(name="stg", bufs=4) as stgp,
            tc.tile_pool(name="ps", bufs=2, space="PSUM") as psS,
            tc.tile_pool(name="dram", bufs=1, space="DRAM") as dram,
        ):
            # ---- constants in SBUF ----
            wqkv_sb = constp.tile([128, KD * 512], BF, tag="wqkv")
            cq = constp.tile([128, TT * 64], BF, tag="cq")
            sq = constp.tile([128, TT * 64], BF, tag="sq")
            ck = constp.tile([128, TT * 64], BF, tag="ck")
            sk = constp.tile([128, TT * 64], BF, tag="sk")
            nc.gpsimd.dma_start(cq[:], cosq)
            nc.gpsimd.dma_start(sq[:], sinq)
            nc.gpsimd.dma_start(ck[:], cosk)
            nc.gpsimd.dma_start(sk[:], sink)
            mask_sb = constp.tile([128, 2048], BF, tag="mask")
            nc.gpsimd.dma_start(mask_sb[:], maskq)
            ident_sb = constp.tile([128, 128], BF, tag="ident")
            nc.gpsimd.dma_start(ident_sb[:], identin)

            # persistent attention operands
            qt_all = persist.tile([128, 4 * T], BF, tag="qt")   # slot=(h*2+b)
            kt_all = persist.tile([128, 2 * T], BF, tag="kt")   # per batch
            vaug = persist.tile([128, 2 * TT * 132], BF, tag="vb")  # per batch

            # A2A bounce buffers (DRAM): per head
            a2a_in = [
                dram.tile([1024, 512], BF, name=f"ain{h}", tag=f"ain{h}")
                for h in range(2)
            ]
            a2a_out = [
                dram.tile([1024, 512], BF, name=f"aout{h}", tag=f"aout{h}")
                for h in range(2)
            ]
            rg = [list(range(NCORES))]

            wo_holder = []

            def load_wo():
                wo_sb = bigp.tile([128, KD * D], BF, tag="big")
                for i in range(KD):
                    nc.sync.dma_start(
                        wo_sb[:, i * D : (i + 1) * D],
                        wo[i * 128 : (i + 1) * 128, :],
                    )
                wo_holder.append(wo_sb)

            cq3 = cq[:].rearrange("p (i u) -> p i u", u=64)
            sq3 = sq[:].rearrange("p (i u) -> p i u", u=64)
            ck3 = ck[:].rearrange("p (i u) -> p i u", u=64)
            sk3 = sk[:].rearrange("p (i u) -> p i u", u=64)

            def _rope_one(b, g, comb3, iu, gs):
                """RoPE + transpose for one 4-tile group / one of q0,q1,k."""
                c3, s3 = (cq3, sq3) if iu < 2 else (ck3, sk3)
                lo = comb3[:, gs, iu * 128 : iu * 128 + 64]
                hi = comb3[:, gs, iu * 128 + 64 : iu * 128 + 128]
                ro = work.tile([128, 512], BF, tag="rope_out", bufs=3)
                ro3 = ro[:].rearrange("p (i u) -> p i u", u=128)
                t1 = work.tile([128, 256], BF, tag="rt1", bufs=2)
                t2_ = work.tile([128, 256], BF, tag="rt2", bufs=2)
                t13 = t1[:].rearrange("p (i u) -> p i u", u=64)
                t23 = t2_[:].rearrange("p (i u) -> p i u", u=64)
                nc.vector.tensor_mul(t13, lo, c3[:, gs, :])
                nc.vector.tensor_mul(t23, hi, s3[:, gs, :])
                nc.vector.tensor_sub(ro3[:, :, 0:64], t13, t23)
                t3 = work.tile([128, 256], BF, tag="rt3", bufs=2)
                t4 = work.tile([128, 256], BF, tag="rt4", bufs=2)
                t33 = t3[:].rearrange("p (i u) -> p i u", u=64)
                t43 = t4[:].rearrange("p (i u) -> p i u", u=64)
                nc.vector.tensor_mul(t33, hi, c3[:, gs, :])
                nc.vector.tensor_mul(t43, lo, s3[:, gs, :])
                nc.vector.tensor_add(ro3[:, :, 64:128], t33, t43)
                if iu < 2:
                    dst, off = qt_all, (iu * 2 + b) * T
                else:
                    dst, off = kt_all, b * T
                tps = psS.tile([128, 512], BF, tag="tp", bufs=2)
                for i in range(4):
                    nc.tensor.transpose(
                        tps[:, i * 128 : (i + 1) * 128],
                        ro[:, i * 128 : (i + 1) * 128],
                        ident_sb[:],
                    )
                nc.any.tensor_copy(
                    dst[:, off + g * 512 : off + (g + 1) * 512], tps[:]
                )

            def rope_q(b, g, comb):
                comb3 = comb[:].rearrange("p (i u) -> p i u", u=512)
                gs = slice(g * 4, (g + 1) * 4)
                _rope_one(b, g, comb3, 0, gs)
                _rope_one(b, g, comb3, 1, gs)

            def rope_kv(b, g, comb):
                comb3 = comb[:].rearrange("p (i u) -> p i u", u=512)
                vb3 = vaug[:, b * TT * 132 : (b + 1) * TT * 132].rearrange(
                    "p (i u) -> p i u", u=132
                )
                gs = slice(g * 4, (g + 1) * 4)
                nc.vector.tensor_copy(vb3[:, gs, 0:128], comb3[:, gs, 384:512])
                nc.vector.memset(vb3[:, gs, 128:129], 1.0)
                _rope_one(b, g, comb3, 2, gs)

            # K/V pair-AllReduce: each core contracts only kd 0..7 for K/V
            # (the host rotates x's feature rows and W's rows by 1024 on
            # odd cores, so the two halves of a core pair cover disjoint
            # halves of the true contraction); a 2-core AllReduce sums the
            # partials. Halves the K/V projection compute.
            KH = KD // 2
            ar_in = [
                dram.tile([128, TT * 256], BF, name=f"arin{b}", tag=f"arin{b}")
                for b in range(2)
            ]
            ar_out = [
                dram.tile([256, TT * 256], BF, name=f"arout{b}", tag=f"arout{b}")
                for b in range(2)
            ]
            rg2 = [[0, 1], [2, 3], [4, 5], [6, 7]]

            def fire_ar(b):
                # pair AllGather (a 2-rank AllReduce moves twice the wire
                # bytes); the partial halves are summed locally on DVE
                nc.gpsimd.collective_compute(
                    "AllGather",
                    mybir.AluOpType.bypass,
                    replica_groups=rg2,
                    ins=[ar_in[b].opt()],
                    outs=[ar_out[b].opt()],
                )

            def projection(b, load_w, dedup):
                """QKV projection for batch b, kd-outer so compute starts
                while x / wqkv stream in. First a kv-partial pass (kd<8,
                fires the pair AllReduce early), then q passes (full kd).
                """
                xt_sb = bigp.tile([128, KD * T], BF, tag="big")
                for kd in range(KD):
                    # alternate HWDGE initiators so the feed isn't
                    # single-queue bound
                    if load_w:
                        weng = nc.scalar if kd % 2 == 0 else nc.sync
                        weng.dma_start(
                            wqkv_sb[:, kd * 512 : (kd + 1) * 512],
                            wqkv[kd * 128 : (kd + 1) * 128, :],
                        )
                    eng = nc.sync if kd % 2 == 0 else nc.scalar
                    eng.dma_start(
                        xt_sb[:, kd * T : (kd + 1) * T],
                        xts[b][kd * 128 : (kd + 1) * 128, :],
                    )
                comb = combp.tile([128, TT * 512], BF, tag="comb")
                comb3 = comb[:].rearrange("p (i u) -> p i u", u=512)

                # Concurrent PSUM accumulation groups must be bank-aligned:
                # a start=True matmul invalidates other in-flight groups in
                # the same 2KB bank. Each 256-col accumulator therefore gets
                # its own bank ([128,1024] tiles hold tiles at cols 0 and
                # 512; [128,512] tiles use cols 0:256 only).
                def _pass_accs(nm):
                    pr = [
                        psS.tile([128, 1024], F32, tag="s2", name=f"{nm}p{a}")
                        for a in range(2)
                    ]
                    sg = [
                        psS.tile([128, 512], F32, tag="av", bufs=2, name=f"{nm}a{a}")
                        for a in range(2)
                    ] + [
                        psS.tile([128, 512], F32, tag="tp", bufs=2, name=f"{nm}t{a}")
                        for a in range(2)
                    ]

                    def tgt(i):
                        if i < 4:
                            return pr[i // 2][:, (i % 2) * 512 : (i % 2) * 512 + 256]
                        return sg[i - 4][:, 0:256]

                    return pr, sg, tgt

                def _pass_copies(pr, sg, base_tt, col0):
                    for a in range(2):
                        src = pr[a][:].rearrange("p (i u) -> p i u", u=512)
                        nc.any.tensor_copy(
                            comb3[:, base_tt + 2 * a : base_tt + 2 * a + 2,
                                  col0 : col0 + 256],
                            src[:, :, 0:256],
                        )
                    for a in range(4):
                        tt = base_tt + 4 + a
                        nc.any.tensor_copy(
                            comb[:, tt * 512 + col0 : tt * 512 + col0 + 256],
                            sg[a][:, 0:256],
                        )

                # ---- kv passes: 2 subpasses of 8 tts ----
                # batch 0 contracts in full locally: the first collective
                # cannot complete before ~85us (rank-sync barrier floor),
                # which would stall attn00; batch 1's AllGather is fully
                # covered by surrounding compute.
                kh = KH if dedup else KD
                for sub in range(2):
                    pr, sg, tgt = _pass_accs(f"kv{b}{sub}")
                    for kd in range(kh):
                        for i in range(8):
                            tt = 8 * sub + i
                            nc.tensor.matmul(
                                tgt(i),
                                xt_sb[:, kd * T + tt * 128 : kd * T + (tt + 1) * 128],
                                wqkv_sb[:, kd * 512 + 256 : (kd + 1) * 512],
                                start=(kd == 0),
                                stop=(kd == kh - 1),
                            )
                    with tc.high_priority():
                        _pass_copies(pr, sg, 8 * sub, 256)
                    if not dedup:
                        with tc.high_priority():
                            rope_kv(b, 2 * sub, comb)
                            rope_kv(b, 2 * sub + 1, comb)
                if dedup:
                    with tc.high_priority():
                        nc.sync.dma_start(
                            ar_in[b][:].rearrange("p (i u) -> p i u", u=256),
                            comb3[:, :, 256:512],
                        )
                    fire_ar(b)

                # ---- q passes: kd 0..15, 2 passes of 8 tts ----
                for p in range(2):
                    pr, sg, tgt = _pass_accs(f"q{b}{p}")
                    for kd in range(KD):
                        for i in range(8):
                            tt = 8 * p + i
                            nc.tensor.matmul(
                                tgt(i),
                                xt_sb[:, kd * T + tt * 128 : kd * T + (tt + 1) * 128],
                                wqkv_sb[:, kd * 512 : kd * 512 + 256],
                                start=(kd == 0),
                                stop=(kd == KD - 1),
                            )
                    _pass_copies(pr, sg, 8 * p, 0)
                    rope_q(b, 2 * p, comb)
                    rope_q(b, 2 * p + 1, comb)

                # ---- gathered kv comes back: per rope-group, sum the two
                # partial halves and rope immediately (pipelines the
                # post-collective latency at 4-tile granularity) ----
                if dedup:
                    ago3 = ar_out[b][0:128, :].rearrange("p (i u) -> p i u", u=256)
                    with tc.high_priority():
                        for g in range(4):
                            sl = slice(4 * g, 4 * g + 4)
                            nc.sync.dma_start(comb3[:, sl, 256:512], ago3[:, sl, :])
                            kvtmp = work.tile([128, 1024], BF, tag="kvtmp", bufs=2)
                            nc.scalar.dma_start(
                                kvtmp[:],
                                ar_out[b][128:256, g * 1024 : (g + 1) * 1024],
                            )
                            nc.vector.tensor_add(
                                comb3[:, sl, 256:512],
                                comb3[:, sl, 256:512],
                                kvtmp[:].rearrange("p (i u) -> p i u", u=256),
                            )
                            rope_kv(b, g, comb)
                if DEBUG_KV and b == 0:
                    dstage = work.tile([128, TT * 256], BF, tag="dbgstage", bufs=1)
                    nc.vector.tensor_copy(
                        dstage[:].rearrange("p (i u) -> p i u", u=256),
                        comb3[:, :, 256:512],
                    )
                    nc.sync.dma_start(dbg_ext, dstage[:])
                return comb

            def attention(h, b):
                """Causal attention for local head h, batch b (S^T form).

                S blocks are produced in j-pairs: both halves of a
                [128,1024] PSUM tile, one exp per pair. Diagonal blocks
                are full-width; the mask multiply zeroes above-diagonal.
                Stages attnout^T [hd, T] quad-wise into a2a_in[h].
                """
                slot = h * 2 + b
                ao_nat = work.tile([128, T], BF, tag="aonat", bufs=1)

                def s_pairs(quad, plo, phi):
                    t0 = quad * 4
                    q0 = slot * T + quad * 512
                    out = []
                    for pi in range(plo, phi):
                        sp = psS.tile([128, 1024], F32, tag="s2")
                        for jh in range(2):
                            j = 2 * pi + jh
                            # diagonal blocks: queries < j*128 are masked
                            # anyway; skip computing them (the stale psum
                            # they leave is exp'd to a finite value, zeroed
                            # by the mask, and never read by AV)
                            c0 = max(j - t0, 0) * 128
                            nc.tensor.matmul(
                                sp[:, jh * 512 + c0 : (jh + 1) * 512],
                                kt_all[:, b * T + j * 128 : b * T + (j + 1) * 128],
                                qt_all[:, q0 + c0 : q0 + 512],
                                start=True,
                                stop=True,
                            )
                        pb = pbp.tile([128, 1024], BF, tag="pb")
                        nc.scalar.activation(
                            pb[:], sp[:],
                            mybir.ActivationFunctionType.Exp,
                            bias=0.0, scale=1.0,
                        )
                        for jh in range(2):
                            j = 2 * pi + jh
                            m = j - t0
                            if m >= 0:
                                nc.vector.tensor_mul(
                                    pb[:, jh * 512 : (jh + 1) * 512],
                                    pb[:, jh * 512 : (jh + 1) * 512],
                                    mask_sb[:, m * 512 : (m + 1) * 512],
                                )
                        out.append(pb)
                    return out

                # pairs per quad = 2*quad + 2
                pairs = {0: s_pairs(0, 0, 2)}
                for quad in range(4):
                    # lookahead: emit next quad's first pairs before this AV
                    if quad < 3:
                        pairs[quad + 1] = s_pairs(quad + 1, 0, 2)
                    for i in range(4):
                        tau = quad * 4 + i
                        avps = psS.tile([128, 132], F32, tag="av", bufs=2)
                        for j in range(tau + 1):
                            pb = pairs[quad][j // 2]
                            c0 = (j % 2) * 512 + i * 128
                            nc.tensor.matmul(
                                avps[:, 0:129],
                                pb[:, c0 : c0 + 128],
                                vaug[
                                    :,
                                    b * TT * 132 + j * 132 : b * TT * 132 + j * 132 + 129,
                                ],
                                start=(j == 0),
                                stop=(j == tau),
                            )
                        r = stgp.tile([128, 1], F32, tag="rc", bufs=4)
                        nc.vector.reciprocal(r[:], avps[:, 128:129])
                        nc.vector.tensor_scalar_mul(
                            ao_nat[:, tau * 128 : (tau + 1) * 128],
                            avps[:, 0:128],
                            r[:],
                        )
                    # transpose this quad's attnout and stage its A2A shard
                    tps = psS.tile([128, 512], BF, tag="tp", bufs=2)
                    for i in range(4):
                        nc.tensor.transpose(
                            tps[:, i * 128 : (i + 1) * 128],
                            ao_nat[:, (quad * 4 + i) * 128 : (quad * 4 + i + 1) * 128],
                            ident_sb[:],
                        )
                    aoq = work.tile([128, 512], BF, tag="atq", bufs=4)
                    nc.any.tensor_copy(aoq[:], tps[:])
                    nc.sync.dma_start(
                        a2a_in[h][(b * 4 + quad) * 128 : (b * 4 + quad + 1) * 128, :],
                        aoq[:],
                    )
                    if quad < 3:
                        pairs[quad + 1].extend(
                            s_pairs(quad + 1, 2, 2 * quad + 4)
                        )
                    del pairs[quad]

            def fire_a2a(h):
                nc.gpsimd.collective_compute(
                    "AllToAll",
                    mybir.AluOpType.bypass,
                    replica_groups=rg,
                    ins=[a2a_in[h].opt()],
                    outs=[a2a_out[h].opt()],
                )

            def load_aos(h):
                aos = []
                for r in range(8):
                    t = aop.tile([128, 512], BF, tag="aotile")
                    nc.sync.dma_start(t[:], a2a_out[h][r * 128 : (r + 1) * 128, :])
                    aos.append(t)
                return aos

            def oproj_half(aos, hh, parts):
                """One half of the out-projection contraction: per output
                tile, accumulate the 8 head blocks of half hh in PSUM.
                The early half (hh=0, whose AllToAll lands first) stages
                its partial to SBUF bf16 so PSUM is never held across the
                second collective; the late half (hh=1) adds the staged
                partial and writes DRAM."""
                wo_sb = wo_holder[0]
                for tt2 in range(4):
                    for dc in range(4):
                        k = tt2 * 4 + dc
                        tag = "av" if (k % 2) == 0 else "tp"
                        ps = psS.tile([128, 512], F32, tag=tag, bufs=2,
                                      name=f"op{hh}_{k}")
                        for r in range(8):
                            head = 2 * r + hh
                            nc.tensor.matmul(
                                ps[:],
                                aos[r][:, tt2 * 128 : (tt2 + 1) * 128],
                                wo_sb[:, head * D + dc * 512 : head * D + (dc + 1) * 512],
                                start=(r == 0),
                                stop=(r == 7),
                            )
                        if hh == 0:
                            part = parts["all"][:, k * 512 : (k + 1) * 512]
                            nc.any.tensor_copy(part, ps[:])
                            parts[k] = part
                        else:
                            stg = stgp.tile([128, 512], F32, tag="ostage", bufs=4)
                            nc.vector.tensor_add(stg[:], ps[:], parts[k])
                            nc.sync.dma_start(
                                out_ext[tt2 * 128 : (tt2 + 1) * 128,
                                        dc * 512 : (dc + 1) * 512],
                                stg[:],
                            )

            # ---- main schedule ----
            # a2a0 fires after attn01 and flies while attn11 runs; a2a1
            # fires after attn11 and flies while the h=0 out-projection
            # half (whose data arrived with a2a0) accumulates + stages.
            PHASE_MARKS.append(("proj0", nc.next_id()))
            with nc.named_scope("proj0"):
                projection(0, load_w=True, dedup=False)
            PHASE_MARKS.append(("attn00", nc.next_id()))
            with nc.named_scope("attn00"):
                attention(0, 0)
            PHASE_MARKS.append(("attn10", nc.next_id()))
            with nc.named_scope("attn10"):
                attention(1, 0)
            PHASE_MARKS.append(("proj1", nc.next_id()))
            with nc.named_scope("proj1"):
                projection(1, load_w=False, dedup=True)
                load_wo()
            PHASE_MARKS.append(("attn01", nc.next_id()))
            with nc.named_scope("attn01"), tc.high_priority():
                attention(0, 1)
            PHASE_MARKS.append(("a2a0", nc.next_id()))
            with nc.named_scope("a2a0"):
                fire_a2a(0)
            aos0 = load_aos(0)
            PHASE_MARKS.append(("attn11", nc.next_id()))
            with nc.named_scope("attn11"):
                attention(1, 1)
            PHASE_MARKS.append(("a2a1", nc.next_id()))
            with nc.named_scope("a2a1"):
                fire_a2a(1)
            aos1 = load_aos(1)
            parts_all = combp.tile([128, 16 * 512], BF, tag="comb", name="parts")
            parts = {"all": parts_all}
            PHASE_MARKS.append(("oproj0", nc.next_id()))
            with nc.named_scope("oproj0"):
                oproj_half(aos0, 0, parts)
            PHASE_MARKS.append(("oproj1", nc.next_id()))
            with nc.named_scope("oproj1"):
                oproj_half(aos1, 1, parts)

    PHASE_MARKS.append(("end", nc.next_id()))
    nc.compile()
    return nc


def _get_compiled():
    global _compiled
    if _compiled is None:
        _compiled = _build()
    return _compiled


def _rope_tables():
    """Natural-layout RoPE tables [128, TT*64] (t-tile-major blocks)."""
    inv_freq = 1.0 / (THETA ** (np.arange(0, HD, 2, dtype=np.float64) / HD))  # [64]
    t = np.arange(T, dtype=np.float64)
    ang = t[:, None] * inv_freq[None, :]          # [T, 64]
    cos = np.cos(ang).astype(np.float32)
    sin = np.sin(ang).astype(np.float32)
    # [T, 64] -> [128, TT*64]: block i columns = rows [128i, 128(i+1))
    cos_n = cos.reshape(TT, 128, 64).transpose(1, 0, 2).reshape(128, TT * 64)
    sin_n = sin.reshape(TT, 128, 64).transpose(1, 0, 2).reshape(128, TT * 64)
    return cos_n, sin_n


def kernel(x, Wq, Wk, Wv, Wo):
    x = np.asarray(x)
    Wq_ = np.asarray(Wq)
    Wk_ = np.asarray(Wk)
    Wv_ = np.asarray(Wv)
    Wo_ = np.asarray(Wo)

    bf = ml_dtypes.bfloat16
    xt = [np.ascontiguousarray(x[b].T).astype(bf) for b in range(B)]
    wo_bf = Wo_.astype(bf)

    cos_n, sin_n = _rope_tables()
    scale = 1.0 / np.sqrt(np.float32(HD))
    cosq = (cos_n * scale).astype(bf)
    sinq = (sin_n * scale).astype(bf)
    cosk = cos_n.astype(bf)
    sink = sin_n.astype(bf)

    kl = np.arange(128)[:, None]
    ql = np.arange(512)[None, :]
    maskq = np.concatenate(
        [(ql >= kl + m * 128).astype(np.float32) for m in range(4)], axis=1
    ).astype(bf)

    in_maps = []
    for c in range(NCORES):
        kv = c // 2
        wqkv = np.concatenate(
            [
                Wq_[:, 2 * c * 128 : (2 * c + 2) * 128],
                Wk_[:, kv * 128 : (kv + 1) * 128],
                Wv_[:, kv * 128 : (kv + 1) * 128],
            ],
            axis=1,
        ).astype(bf)
        # odd cores see the contraction dim rotated by 1024 so that the
        # uniform "kd<8" K/V partial covers the other half of the true
        # contraction; x@W is invariant under the matched row rotation.
        if c % 2:
            wqkv = np.roll(wqkv, -1024, axis=0)
        in_maps.append(
            {
                "xt0": xt[0] if c % 2 == 0 else np.roll(xt[0], -1024, axis=0),
                "xt1": xt[1] if c % 2 == 0 else np.roll(xt[1], -1024, axis=0),
                "wqkv": wqkv,
                "wo": wo_bf,
                "cosq": cosq,
                "sinq": sinq,
                "cosk": cosk,
                "sink": sink,
                "maskq": maskq,
                "identin": np.eye(128, dtype=np.float32).astype(bf),
            }
        )

    nc = _get_compiled()
    global LAST_RESULT
    kw = {}
    if TRACE:
        kw = dict(trace=True, tmpdir=TRACE_DIR)
    try:
        res = run_bass_kernel_spmd(nc, in_maps, list(range(NCORES)), **kw)
    except Exception:
        # transient NRT_EXEC_UNIT_UNRECOVERABLE has been observed once per
        # session on this fleet; one retry clears it
        import time as _time

        _time.sleep(10)
        res = run_bass_kernel_spmd(nc, in_maps, list(range(NCORES)), **kw)
    LAST_RESULT = res
    out = np.empty((B * T, D), dtype=np.float32)
    for c in range(NCORES):
        out[c * 512 : (c + 1) * 512, :] = res.results[c]["out"]
    return out.reshape(B, T, D)
